# revision 15
# baseline (speedup 1.0000x reference)
"""Trainium2 Bass kernel for nn_MultiHeadAttention_8546984919667.

B=1, S=4096, D_MODEL=1024, H=16 heads, Dk=64.
Sharding: tensor-parallel over heads — each of the 8 cores owns 2 heads
(a 128-wide slice of the q/k/v projection outputs and of Wo's columns),
computes full attention for those heads, and produces a partial output
projection [S, D]. Host sums the 8 partials and adds bo.

On-device dataflow (all matmuls bf16 operands, f32 PSUM accumulation):
  A) Q^T, K^T, V^T [128, S] = W_x^T-chunks.T @ x^T-chunks (+bias per
     partition).  K^T is stored zero-padded per head (other head's 64
     rows zeroed) so the scores matmuls contract over K=128 and stream
     at full PE rate.  V^T is PE-transposed into V-natural blocks with
     an appended ones column (softmax denominator trick).
  B) per query block of 512: scores^T [128t, 512s] = KTz_h-tile.T @
     Q^T, one Exp per two t-chunks ([128, 1024] over a 2-bank PSUM
     tile), ctx^T accumulated over the 32 t-chunks; PSUM row 64 ends up
     holding the softmax denominator.
  C) per query block: reciprocal + ones-outer-product broadcast +
     normalize, then out_partial [512, 1024] = ctx^T-slices.T @
     Wo^T-slice, DMA to HBM.  Emitted per-block so the scheduler can
     slot this PE work under the next block's exps (keeps the PE dense —
     an idle PE gets clock-throttled to half rate on this part).
"""

import sys

if "/opt/trn_rl_repo" not in sys.path:
    sys.path.insert(0, "/opt/trn_rl_repo")

import numpy as np
import ml_dtypes

import concourse.bass as bass
import concourse.tile as tile
from concourse import mybir
from concourse.bass_utils import run_bass_kernel_spmd

BF16 = ml_dtypes.bfloat16
F32 = mybir.dt.float32
BF = mybir.dt.bfloat16

S = 4096          # sequence length
D = 1024          # d_model
N_CORES = 8
DK = 64           # head dim
HPC = 2           # heads per core
PC = 128          # projection slice per core (HPC * DK)
NCH = D // 128    # 8 contraction chunks of 128
SB = 512          # query-block width (PSUM bank)
NSB = S // SB     # 8 query blocks
SPW = 2048        # projection block width (4KB DMA rows, 4 query blocks)
NSP = S // SPW    # 2 projection blocks
NT = S // 128     # 32 key/value chunks
AUG = DK + 1      # V block width with ones column

LAST_RESULT = None  # test harness reads exec_time_ns from here


def _split_multi_waits(nc):
    """This walrus build allows only one sync wait per instruction; move
    extras onto preceding same-engine NoOps."""
    for fn in nc.m.functions:
        for blk in fn.blocks:
            new_insts = []
            for ins in blk.instructions:
                si = ins.sync_info
                if si is not None and si.on_wait and len(si.on_wait) > 1:
                    extra = list(si.on_wait[:-1])
                    si.on_wait = [si.on_wait[-1]]
                    for j, w in enumerate(extra):
                        new_insts.append(mybir.InstNoOp(
                            name=f"{ins.name}-wsplit{j}",
                            engine=ins.engine,
                            ins=[], outs=[],
                            sync_info=mybir.SyncInfo(on_wait=[w], on_update=[]),
                        ))
                new_insts.append(ins)
            blk.instructions = new_insts


def _build():
    nc = bass.Bass("TRN2", target_bir_lowering=False, debug=False,
                   num_devices=N_CORES)

    qT = nc.dram_tensor("qT", [D, S], BF, kind="ExternalInput").ap()
    kT = nc.dram_tensor("kT", [D, S], BF, kind="ExternalInput").ap()
    vT = nc.dram_tensor("vT", [D, S], BF, kind="ExternalInput").ap()
    wq = nc.dram_tensor("wq", [D, PC], BF, kind="ExternalInput").ap()
    wk = nc.dram_tensor("wk", [D, PC], BF, kind="ExternalInput").ap()
    wv = nc.dram_tensor("wv", [D, PC], BF, kind="ExternalInput").ap()
    bqd = nc.dram_tensor("bqd", [PC, 1], F32, kind="ExternalInput").ap()
    bkd = nc.dram_tensor("bkd", [PC, 1], F32, kind="ExternalInput").ap()
    bvd = nc.dram_tensor("bvd", [PC, 1], F32, kind="ExternalInput").ap()
    wo = nc.dram_tensor("wo", [PC, D], BF, kind="ExternalInput").ap()
    ident = nc.dram_tensor("ident", [128, 128], BF, kind="ExternalInput").ap()
    out = nc.dram_tensor("out", [S, D], F32, kind="ExternalOutput").ap()

    with tile.TileContext(nc) as tc:
        with (
            tc.tile_pool(name="persist", bufs=1) as persist,
            tc.tile_pool(name="xin", bufs=8) as xin,
            tc.tile_pool(name="ep", bufs=6) as ep,
            tc.tile_pool(name="op", bufs=4) as op,
            tc.tile_pool(name="ps", bufs=3, space="PSUM") as ps,
            tc.tile_pool(name="psc", bufs=2, space="PSUM") as psc,
        ):
            # ---- persistent SBUF tensors ----
            QTs = [persist.tile([PC, SPW], BF, tag=f"QT{i}", name=f"QT{i}")
                   for i in range(NSP)]
            # zero-padded K^T per head: full-rate K=128 scores matmuls
            KTz = [persist.tile([PC, S], BF, tag=f"KTz{h}", name=f"KTz{h}")
                   for h in range(HPC)]
            VT = persist.tile([PC, S], BF, tag="VT")
            VnA = persist.tile([PC, HPC * NT * AUG], BF, tag="VnA")
            ctxuT = persist.tile([PC, S], F32, tag="ctxuT")
            ctxT = persist.tile([PC, S], BF, tag="ctxT")
            rden = persist.tile([1, HPC * S], F32, tag="rden")
            w_q = persist.tile([128, D], BF, tag="w_q")
            w_k = persist.tile([128, D], BF, tag="w_k")
            w_v = persist.tile([128, D], BF, tag="w_v")
            w_o = persist.tile([PC, D], BF, tag="w_o")
            bq_s = persist.tile([PC, 1], F32, tag="bq_s")
            bk_s = persist.tile([PC, 1], F32, tag="bk_s")
            bv_s = persist.tile([PC, 1], F32, tag="bv_s")
            ones_f = persist.tile([1, DK], F32, tag="ones_f")
            id_s = persist.tile([128, 128], BF, tag="id_s")

            # ---- load weights / constants ----
            for wtile, wdram in ((w_q, wq), (w_k, wk), (w_v, wv)):
                nc.sync.dma_start(
                    wtile[:].rearrange("p (c n) -> p c n", c=NCH),
                    wdram.rearrange("(c p) n -> p c n", c=NCH),
                )
            nc.sync.dma_start(w_o[:], wo[:, :])
            nc.sync.dma_start(bq_s[:], bqd[:, :])
            nc.sync.dma_start(bk_s[:], bkd[:, :])
            nc.sync.dma_start(bv_s[:], bvd[:, :])
            nc.sync.dma_start(id_s[:], ident[:, :])
            nc.gpsimd.memset(ones_f[:], 1.0)
            # ones columns of the augmented V blocks
            nc.gpsimd.memset(VnA[:], 1.0)
            nc.gpsimd.memset(KTz[0][DK:PC, :], 0.0)
            nc.gpsimd.memset(KTz[1][0:DK, :], 0.0)
            # preload the ACT exp table early so the first real exp doesn't
            # stall the B-phase pipeline
            warm = persist.tile([128, 8], F32, tag="warm")
            nc.gpsimd.memset(warm[:], 0.0)
            nc.scalar.activation(warm[:, 4:8], warm[:, 0:4],
                                 mybir.ActivationFunctionType.Exp, scale=1.0)

            # ---- phase A ----
            with nc.allow_low_precision(reason="bf16 activations by design"):
                def proj_psum(xdram, wtile, sp):
                    # 4KB HBM rows per DMA keep the HWDGE descriptor count
                    # (and SP trigger time) low
                    xts = []
                    for ch in range(NCH):
                        xt = xin.tile([128, SPW], BF, tag="xt")
                        nc.sync.dma_start(
                            xt[:],
                            xdram[ch * 128:(ch + 1) * 128, bass.ts(sp, SPW)],
                        )
                        xts.append(xt)
                    pts = [ps.tile([128, 2 * SB], F32, tag="ps", name=f"pt{sp}_{i}")
                           for i in range(2)]
                    for ch in range(NCH):
                        for q in range(4):
                            qsl = slice(q * SB, (q + 1) * SB)
                            osl = slice((q % 2) * SB, (q % 2 + 1) * SB)
                            nc.tensor.matmul(
                                pts[q // 2][:, osl], wtile[:, bass.ts(ch, 128)],
                                xts[ch][:, qsl],
                                start=(ch == 0), stop=(ch == NCH - 1),
                            )
                    return pts

                for sp in range(NSP):
                    pts = proj_psum(kT, w_k, sp)
                    for i in range(2):
                        csl = slice(sp * SPW + i * 2 * SB,
                                    sp * SPW + (i + 1) * 2 * SB)
                        for h in range(HPC):
                            hs = slice(h * DK, (h + 1) * DK)
                            nc.vector.tensor_scalar_add(
                                KTz[h][hs, csl], pts[i][hs, :],
                                bk_s[hs, 0:1],
                            )

                for sp in range(NSP):
                    pts = proj_psum(vT, w_v, sp)
                    for i in range(2):
                        csl = slice(sp * SPW + i * 2 * SB,
                                    sp * SPW + (i + 1) * 2 * SB)
                        nc.vector.tensor_scalar_add(
                            VT[:, csl], pts[i][:], bv_s[:, 0:1])
                    # transpose V^T -> V natural blocks (ones columns already
                    # memset in VnA)
                    for tt in range(sp * (SPW // 128), (sp + 1) * (SPW // 128)):
                        ptt = ps.tile([128, 128], BF, tag="ps")
                        nc.tensor.transpose(
                            ptt[:], VT[:, bass.ts(tt, 128)], id_s[:])
                        for h in range(HPC):
                            base = (h * NT + tt) * AUG
                            nc.vector.tensor_copy(
                                VnA[:, base:base + DK],
                                ptt[:, h * DK:(h + 1) * DK])

                def emit_qproj(sp):
                    pts = proj_psum(qT, w_q, sp)
                    for i in range(2):
                        nc.vector.tensor_scalar_add(
                            QTs[sp][:, i * 2 * SB:(i + 1) * 2 * SB],
                            pts[i][:], bq_s[:, 0:1])

                emit_qproj(0)

                # ---- phase B/C: attention per query block ----
                def emit_norm_and_out(sb):
                    # normalize query block sb and project it to the output.
                    # Emitted mid-way through the NEXT block's t-loop so this
                    # PE work has lower scheduling priority than the scores
                    # feeding ACT — it fills PE gaps instead of starving ACT.
                    bps = ps.tile([PC, SB], F32, tag="ps", name=f"bps{sb}")
                    for h in range(HPC):
                        nc.tensor.matmul(
                            bps[h * DK:(h + 1) * DK, :], ones_f[0:1, :],
                            rden[0:1, h * S + sb * SB:h * S + (sb + 1) * SB],
                            start=True, stop=True,
                        )
                    nc.vector.tensor_mul(
                        ctxT[:, bass.ts(sb, SB)],
                        ctxuT[:, bass.ts(sb, SB)], bps[:],
                    )
                    for st in range(4 * sb, 4 * (sb + 1)):
                        po = ps.tile([128, 2 * SB], F32, tag="ps",
                                     name=f"po{st}")
                        lhs = ctxT[:, bass.ts(st, 128)]
                        nc.tensor.matmul(po[:, 0:SB], lhs, w_o[:, 0:SB],
                                         start=True, stop=True)
                        nc.tensor.matmul(po[:, SB:D], lhs, w_o[:, SB:D],
                                         start=True, stop=True)
                        ot = op.tile([128, D], F32, tag="ot", name=f"ot{st}")
                        nc.vector.tensor_copy(ot[:, 0:SB], po[:, 0:SB])
                        nc.vector.tensor_copy(ot[:, SB:D], po[:, SB:D])
                        nc.sync.dma_start(out[bass.ts(st, 128), :], ot[:])

                # second Q-projection block only gates query blocks 4-7;
                # defer it so its DMAs+matmuls overlap the start of phase B
                pending = [lambda: emit_qproj(1)]
                for sb in range(NSB):
                    qrhs = QTs[sb // 4][:, (sb % 4) * SB:(sb % 4 + 1) * SB]
                    for h in range(HPC):
                        hs = slice(h * DK, (h + 1) * DK)
                        cps = psc.tile([AUG, SB], F32, tag="psc")
                        for tp in range(NT // 2):
                            # two t-chunks share one 2-bank PSUM tile so a
                            # single Exp covers both (halves ACT overhead)
                            sps = ps.tile([128, 2 * SB], F32, tag="ps")
                            for half in range(2):
                                tt = 2 * tp + half
                                nc.tensor.matmul(
                                    sps[:, half * SB:(half + 1) * SB],
                                    KTz[h][:, bass.ts(tt, 128)],
                                    qrhs,
                                    start=True, stop=True,
                                )
                            et = ep.tile([128, 2 * SB], BF, tag="et")
                            nc.scalar.activation(
                                et[:], sps[:],
                                mybir.ActivationFunctionType.Exp, scale=0.125,
                            )
                            for half in range(2):
                                tt = 2 * tp + half
                                base = (h * NT + tt) * AUG
                                nc.tensor.matmul(
                                    cps[:], VnA[:, base:base + AUG],
                                    et[:, half * SB:(half + 1) * SB],
                                    start=(tt == 0), stop=(tt == NT - 1),
                                )
                            if h == 0 and tp == 2 and pending:
                                for thunk in pending:
                                    thunk()
                                pending = []
                        nc.vector.reciprocal(
                            rden[0:1, h * S + sb * SB:h * S + (sb + 1) * SB],
                            cps[DK:AUG, :])
                        nc.vector.tensor_copy(
                            ctxuT[hs, bass.ts(sb, SB)], cps[0:DK, :])

                    pending.append(lambda sb=sb: emit_norm_and_out(sb))
                for thunk in pending:
                    thunk()

    return nc


_NC = None


def _get_nc():
    global _NC
    if _NC is None:
        _NC = _build()
        _split_multi_waits(_NC)
    return _NC


def kernel(q, k, v, Wq, bq, Wk, bk, Wv, bv, Wo, bo):
    global LAST_RESULT
    nc = _get_nc()

    q2, k2, v2 = (np.asarray(x, np.float32)[0] for x in (q, k, v))
    qTh = np.ascontiguousarray(q2.T).astype(BF16)
    kTh = np.ascontiguousarray(k2.T).astype(BF16)
    vTh = np.ascontiguousarray(v2.T).astype(BF16)
    identh = np.eye(128, dtype=BF16)

    in_maps = []
    for c in range(N_CORES):
        sl = slice(c * PC, (c + 1) * PC)
        in_maps.append({
            "qT": qTh, "kT": kTh, "vT": vTh,
            "wq": np.ascontiguousarray(np.asarray(Wq, np.float32)[sl].T).astype(BF16),
            "wk": np.ascontiguousarray(np.asarray(Wk, np.float32)[sl].T).astype(BF16),
            "wv": np.ascontiguousarray(np.asarray(Wv, np.float32)[sl].T).astype(BF16),
            "bqd": np.asarray(bq, np.float32)[sl].reshape(PC, 1).copy(),
            "bkd": np.asarray(bk, np.float32)[sl].reshape(PC, 1).copy(),
            "bvd": np.asarray(bv, np.float32)[sl].reshape(PC, 1).copy(),
            "wo": np.ascontiguousarray(np.asarray(Wo, np.float32)[:, sl].T).astype(BF16),
            "ident": identh,
        })

    res = run_bass_kernel_spmd(nc, in_maps, core_ids=list(range(N_CORES)))
    LAST_RESULT = res

    acc = np.zeros((S, D), np.float32)
    for c in range(N_CORES):
        acc += res.results[c]["out"]
    acc += np.asarray(bo, np.float32)[None, :]
    return acc[None].astype(np.float32)
